# revision 4
# baseline (speedup 1.0000x reference)
"""Trainium2 Bass kernel for CrossDecoder kNN-mining margin loss (fp8 v2).

Math: mine, per query q (both columns of train_ill), the k+1 nearest rows of
X (rows = concat of both manifolds, dim 512) and use the distances from q to
its own k nearest neighbours (self excluded) in a margin loss.  Rank by
s'(q,j) = 2 q.y_j - (|y_j|^2 - 512)  (descending); recover
dist = |q|^2 + 512 - s' on the host.

Device pipeline (SPMD over 8 cores, candidates sharded 30000 -> 3750/core):
  - PE: fp8(e4m3) DoubleRow matmuls (K=256/mm, 2 mms per accumulation group)
    write raw 2q.y scores for a [128-query, 470-candidate] chunk into PSUM.
    Chunks live at 512-f32 strides so each chunk owns one PSUM bank; 2 groups
    (A=banks 0-3, B=banks 4-7) single-buffered: while PE fills B, ACT drains A.
  - ACT: drains each 4-chunk group PSUM fp32 -> SBUF fp16 (raw scores).
  - GPSIMD (Pool): subtracts the per-candidate bias (|y|^2 - 512, fp16).
  - DVE: one max8 per group: top-8 of 1880 biased scores.
Each core emits, per query, 2 groups x top-8 = 16 candidate scores.
Host merges 8 cores x 16 = 128 candidates/row -> top-(k+1) (fails only if
>8 of the true top-11 land in one 1880-wide group: P ~ 1e-7).
"""

import os
import numpy as np

M_, N_, D_, T_ = 2, 30000, 256, 3000
NCORES = 8
NSHARD = N_ // NCORES          # 3750
FCH = 470                      # candidate chunk width (scores computed)
FST = 512                      # chunk stride in PSUM (one 2KB fp32 bank)
NFC = 8                        # chunks per shard
NPAD = FCH * NFC               # 3760
GW = 4 * FCH                   # 1880 candidates per drain group
KD = M_ * D_                   # 512 contraction dim
KCH = KD // 128                # 4 k-slots (2 DoubleRow matmuls)
QT = 128                       # queries per tile (PSUM partition dim)
NQ = 6016                      # both query sets packed (6000) padded, 47 tiles
NQT = NQ // QT                 # 47
QBLK = 4                       # query tiles per DMA block
NBLK = 12                      # 11 full blocks + one 3-tile block
PAD_BIAS = 60000.0             # pad candidates rank last (fits fp16)
BOFF = 512.0                   # bias offset keeping fp16 |values| small

_cache = {}


def _build_program():
    import concourse.bass as bass
    import concourse.tile as tile
    from concourse import bacc, mybir

    dt = mybir.dt
    DR = mybir.MatmulPerfMode.DoubleRow
    nc = bacc.Bacc(
        "TRN2", target_bir_lowering=False, debug=False, num_devices=NCORES
    )

    xq_d = nc.dram_tensor("xq", [128, KCH, NQ], dt.float8e4, kind="ExternalInput")
    xs_d = nc.dram_tensor("xs", [128, KCH, NPAD], dt.float8e4, kind="ExternalInput")
    bias_d = nc.dram_tensor("bias", [128, NFC, FCH], dt.float16,
                            kind="ExternalInput")
    cand_d = nc.dram_tensor("cand", [NBLK, 128, QBLK * 16], dt.float16,
                            kind="ExternalOutput")

    with tile.TileContext(nc) as tc:
        with (
            tc.tile_pool(name="resident", bufs=1) as res_pool,
            tc.tile_pool(name="xq", bufs=2) as xq_pool,
            tc.tile_pool(name="raw", bufs=2) as raw_pool,
            tc.tile_pool(name="sc", bufs=2) as sc_pool,
            tc.tile_pool(name="cand", bufs=2) as cand_pool,
            tc.tile_pool(name="psum", bufs=1, space=bass.MemorySpace.PSUM) as psum_pool,
        ):
            xs_sb = res_pool.tile([128, KCH, NPAD], dt.float8e4, tag="xs")
            nc.sync.dma_start(out=xs_sb[:, :, :], in_=xs_d[:, :, :])
            bias_sb = res_pool.tile([128, NFC, FCH], dt.float16, tag="bias")
            nc.sync.dma_start(out=bias_sb[:, :, :], in_=bias_d[:, :, :])

            # 2 PSUM groups x 4 banks; single-buffered ping-pong
            ps = [psum_pool.tile([128, 4, FST], dt.float32, tag=f"ps{g}",
                                 name=f"ps{g}")
                  for g in range(2)]

            for blk in range(NBLK):
                q0 = blk * QBLK * QT
                nqt = min(QBLK, NQT - blk * QBLK)
                xq_sb = xq_pool.tile([128, KCH, nqt * QT], dt.float8e4, tag="xq")
                nc.sync.dma_start(out=xq_sb[:, :, :],
                                  in_=xq_d[:, :, q0:q0 + nqt * QT])
                cand_sb = cand_pool.tile([128, nqt * 16], dt.float16, tag="cand")
                for j in range(nqt):
                    for g in range(2):
                        for i in range(2):          # K halves (DoubleRow)
                            for fi in range(4):     # chunks in group
                                f = g * 4 + fi
                                nc.tensor.matmul(
                                    ps[g][:, fi, 0:FCH],
                                    lhsT=xq_sb[:, 2*i:2*i+2, j*QT:(j+1)*QT],
                                    rhs=xs_sb[:, 2*i:2*i+2, f*FCH:(f+1)*FCH],
                                    start=(i == 0),
                                    stop=(i == 1),
                                    perf_mode=DR,
                                )
                        raw_g = raw_pool.tile([128, 4, FCH], dt.float16,
                                              tag=f"raw{g}")
                        nc.scalar.activation(
                            raw_g[:, :, :], ps[g][:, 0:4, 0:FCH],
                            mybir.ActivationFunctionType.Copy,
                        )
                        sc_g = sc_pool.tile([128, 4, FCH], dt.float16,
                                            tag=f"sc{g}")
                        nc.gpsimd.tensor_sub(
                            sc_g[:, :, :], raw_g[:, :, :],
                            bias_sb[:, g*4:(g+1)*4, :],
                        )
                        o = j * 16 + g * 8
                        nc.vector.max(cand_sb[:, o:o + 8], sc_g[:, :, :])
                nc.sync.dma_start(out=cand_d[blk, :, :nqt * 16],
                                  in_=cand_sb[:, :])

    nc.compile()
    return nc


def _get_program():
    if "nc" not in _cache:
        _cache["nc"] = _build_program()
    return _cache["nc"]


def _to_dr_layout(A):
    """[n, 512] fp8 -> [128, 4, n]: out[p, s, :] = A[:, s*128+p]."""
    return np.ascontiguousarray(A.T.reshape(KCH, 128, -1).transpose(1, 0, 2))


def _prep_inputs(X, left, right):
    """X: [N, 512] fp32. Returns per-core input maps."""
    import ml_dtypes

    f8 = ml_dtypes.float8_e4m3
    q_idx = np.concatenate([right, left, np.zeros(NQ - 2 * T_, np.int64)])
    Xq8 = (2.0 * X[q_idx]).astype(f8)
    Xq8[2 * T_:] = 0.0
    xq_in = _to_dr_layout(Xq8)

    per_core = []
    for corei in range(NCORES):
        shard = X[corei * NSHARD:(corei + 1) * NSHARD]          # [3750, 512]
        xs8 = np.zeros((NPAD, KD), f8)
        xs8[:NSHARD] = shard.astype(f8)
        bias = np.full(NPAD, PAD_BIAS, np.float32)
        bias[:NSHARD] = (shard.astype(np.float64) ** 2).sum(1) - BOFF
        bias_t = np.broadcast_to(
            bias.astype(np.float16).reshape(NFC, FCH), (128, NFC, FCH))
        per_core.append({
            "xq": xq_in,
            "xs": _to_dr_layout(xs8),
            "bias": np.ascontiguousarray(bias_t),
        })
    return per_core


def _mine_scores(in_maps, trace=False):
    from concourse.bass_utils import run_bass_kernel_spmd

    nc = _get_program()
    try:
        res = run_bass_kernel_spmd(nc, in_maps, list(range(NCORES)), trace=trace)
    except Exception:
        if not trace:
            raise
        res = run_bass_kernel_spmd(nc, in_maps, list(range(NCORES)), trace=False)
    _cache["last_result"] = res
    # per-core cand: [NBLK, 128, QBLK*16] -> [NQ, 16]
    cores = []
    for i in range(NCORES):
        c = res.results[i]["cand"].reshape(NBLK, 128, QBLK, 16)
        cores.append(c.transpose(0, 2, 1, 3).reshape(NBLK * QBLK * 128, 16)[:NQ])
    return np.concatenate(cores, axis=1).astype(np.float32)     # [NQ, 128]


def kernel(outlayer, c, train_ill, k):
    k = int(k)
    outlayer = np.asarray(outlayer, np.float32)
    train_ill = np.asarray(train_ill)
    X = np.ascontiguousarray(
        outlayer.transpose(1, 0, 2).reshape(N_, KD)).astype(np.float32)
    left = train_ill[:, 0].astype(np.int64)
    right = train_ill[:, 1].astype(np.int64)

    in_maps = _prep_inputs(X, left, right)
    scores = _mine_scores(in_maps, trace=bool(int(os.environ.get("KNN_TRACE", "0"))))

    # top-(k+1) scores (descending) per query row; row 0 is the self match.
    nkeep = k + 1
    part = np.partition(scores, scores.shape[1] - nkeep, axis=1)[:, -nkeep:]
    top = np.sort(part, axis=1)[:, ::-1]                         # [NQ, k+1]

    X64 = X.astype(np.float64)
    sq = (X64 ** 2).sum(1)                                       # [N]

    s_right = top[:T_]                                           # mining of right idx
    s_left = top[T_:2 * T_]                                      # mining of left idx

    # B[i, j] = dist(q_i, j-th NN of q_i) = |q_i|^2 + BOFF - s', self dropped
    B2 = sq[right][:, None] + BOFF - s_right[:, 1:].astype(np.float64)
    B1 = sq[left][:, None] + BOFF - s_left[:, 1:].astype(np.float64)

    D = ((X64[left] - X64[right]) ** 2).sum(1) + 1.0             # [t]
    L1 = np.maximum(D[:, None] - B1, 0.0)
    L2 = np.maximum(D[:, None] - B2, 0.0)
    loss = (L1.mean() + L2.mean()) / 2.0
    return np.asarray(loss, dtype=np.float32)
